# revision 8
# baseline (speedup 1.0000x reference)
"""Trainium2 Bass kernel for nn_CATBlock (IPSA window attention + DWConv MLP).

Sharding: data-parallel over batch B=8, one image per NeuronCore.

Per-core dataflow (T = 112*112 = 12544 tokens, C = 192):
  Stage A (16 blocks of 7 image rows = 784 tokens = 16 windows of 7x7):
    LN1 (token-major, stats on DVE) -> transpose to channel-first (PE),
    evacuated window-major into fp8 (both K-chunks in one [97, 2, *] tile
    so fp8 DoubleRow matmuls contract 194 rows per pass; weights carry a
    x32 pre-scale to stay in e4m3 normal range, descaled at PSUM evac) ->
    QKV DoubleRow matmuls -> per-head window attention (scores batched 4
    heads per matmul via a 3-dim rhs AP, softmax via Exp on ACT +
    multiplicative exp(bias) + ones-matmul denominators) -> AV (PE) ->
    proj (DoubleRow over the two 128-row strip chunks) -> transpose back
    token-major -> residual add -> x1 to DRAM.
  Stage B (16 blocks with a 1-row halo for the 3x3 depthwise conv):
    LN2 -> fc1 DoubleRow (192->768, fp8) -> depthwise 3x3 conv as 5
    accumulating DoubleRow matmuls (tap pairs packed into the fp8 2-tile)
    -> Gelu (conv bias + 1/32 descale folded into the activation) ->
    fc2 DoubleRow (768->192, bias folded into the evacuation) ->
    transpose token-major -> residual add -> out.
"""

import os
import sys

sys.path.insert(0, "/opt/trn_rl_repo")

import numpy as np

import concourse.bass as bass
import concourse.tile as tile
from concourse import mybir
from concourse import library_config
from concourse.bass_utils import run_bass_kernel_spmd
from concourse.vector_clock import ScopedClock

# ---------------------------------------------------------------------------
# Workaround: walrus codegen rejects CTRL/Drain instructions carrying more
# than a couple of semaphore waits ("Too many sync wait commands").  Split the
# TileContext tail drain's waits across sequential drains on SP.
# ---------------------------------------------------------------------------


def _drain_and_barrier(self, tick_clock, wait_clock):
    drain_inst = self.nc.sync.drain()
    wait_clock.add_sem_waits(
        drain_inst.ins, ScopedClock({None: tick_clock.global_clock})
    )
    si = drain_inst.ins.sync_info
    if si is not None and len(si.on_wait) > 1:
        waits = list(si.on_wait)
        drain_inst.ins.sync_info = mybir.SyncInfo(
            on_wait=waits[:1], on_update=list(si.on_update)
        )
        for k in range(1, len(waits)):
            d2 = self.nc.sync.drain()
            d2.ins.sync_info = mybir.SyncInfo(on_wait=[waits[k]], on_update=[])

    self.nc.all_engine_barrier()
    assert self.sems is not None
    popped = self.nc._tile_sem_poison_stack.pop()
    assert popped is self._sem_poison
    self.nc.clear_and_free_semaphores(list(self.sems.allocated().values()))
    self.nc.all_engine_barrier()


tile.TileContext._drain_and_barrier = _drain_and_barrier

_MAX_WAITS = int(os.environ.get("BASSK_MAXW", 1))
_split_ctr = [0]


def _split_excess_waits(nc):
    """Walrus codegen caps the semaphore-wait slots per instruction; move
    excess waits onto preceding same-engine no-ops."""
    for fn in nc.m.functions:
        for bb in fn.blocks:
            insts = bb.instructions
            i = 0
            while i < len(insts):
                inst = insts[i]
                si = getattr(inst, "sync_info", None)
                if si is not None and len(si.on_wait) > _MAX_WAITS:
                    waits = list(si.on_wait)
                    inst.sync_info = mybir.SyncInfo(
                        on_wait=waits[:_MAX_WAITS], on_update=list(si.on_update)
                    )
                    extra = waits[_MAX_WAITS:]
                    for k in range(0, len(extra), _MAX_WAITS):
                        _split_ctr[0] += 1
                        nop = mybir.InstNoOp(
                            name=f"waitsplit-{_split_ctr[0]}", ins=[], outs=[]
                        )
                        nop.engine = inst.engine
                        nop.sync_info = mybir.SyncInfo(
                            on_wait=extra[k:k + _MAX_WAITS], on_update=[]
                        )
                        insts.insert(i, nop)
                        i += 1
                i += 1


# ---------------------------------------------------------------------------

PS = 7
C = 192
HEADS = 8
HD = 24
HID = 768
B = 8
HH = 112
WW = 112
EPS = 1e-5
T = HH * WW          # 12544 tokens per image
NBLK = 16            # window-row blocks per image
BT = PS * WW         # 784 tokens per block
NW = WW // PS        # 16 windows per block
N = PS * PS          # 49 tokens per window

f32 = mybir.dt.float32
bf16 = mybir.dt.bfloat16
fp8 = mybir.dt.float8e4

AL = mybir.AluOpType
AF = mybir.ActivationFunctionType
DR = mybir.MatmulPerfMode.DoubleRow

WS = 32.0            # fp8 weight pre-scale
WSI = 1.0 / WS

# depthwise tap pairs: tap index = 3*(di+1)+(dj+1); the second element of
# pair 4 is a zero diagonal (9 taps -> 5 DoubleRow matmuls).  The rhs AP's
# 2-dim stride is the element delta between the paired taps' shifted views
# of h_sb [128, 9, 114].
DW_PAIRS = [(0, 1), (2, 3), (4, 5), (6, 7), (8, 8)]


def _tap_off(tap):
    di, dj = tap // 3 - 1, tap % 3 - 1
    return di * 114 + dj


def _rel_pos_index(ps):
    coords = np.stack(np.meshgrid(np.arange(ps), np.arange(ps), indexing="ij"))
    cf = coords.reshape(2, -1)
    rel = (cf[:, :, None] - cf[:, None, :]).transpose(1, 2, 0).astype(np.int64)
    rel[:, :, 0] += ps - 1
    rel[:, :, 1] += ps - 1
    rel[:, :, 0] *= 2 * ps - 1
    return rel.sum(-1)


def prep_consts(i):
    """Host-side constant preparation (all tiny).  Returns dict of numpy
    arrays keyed by the kernel's ExternalInput names."""
    g1 = i["norm1_g"].astype(np.float64)
    b1 = i["norm1_b"].astype(np.float64)
    g2 = i["norm2_g"].astype(np.float64)
    b2 = i["norm2_b"].astype(np.float64)
    q_w = i["q_w"].astype(np.float64)
    kv_w = i["kv_w"].astype(np.float64)
    scale = HD ** -0.5

    wq = q_w * g1[None, :] * scale
    bq = (q_w @ b1 + i["q_b"].astype(np.float64)) * scale
    wk = kv_w[:C] * g1[None, :]
    bk = kv_w[:C] @ b1 + i["kv_b"].astype(np.float64)[:C]
    wv = kv_w[C:] * g1[None, :]
    bv = kv_w[C:] @ b1 + i["kv_b"].astype(np.float64)[C:]

    # qk lhsT: [K, M].  Heads padded to 32-partition strips; K split into two
    # 97-row DoubleRow tiles (row 96 of tile0 = bias, matched by the ones row
    # on the activations; row 96 of tile1 = 0).
    def head_pad(w_t):
        o = np.zeros((w_t.shape[0], 2, 128))
        for h2 in range(2):
            for hl in range(4):
                o[:, h2, 32 * hl:32 * hl + HD] = \
                    w_t[:, 96 * h2 + HD * hl:96 * h2 + HD * (hl + 1)]
        return o

    wq_p = head_pad(wq.T)                                 # [192, 2, 128]
    wk_p = head_pad(wk.T)
    bq_p = head_pad(bq[None, :])                          # [1, 2, 128]
    bk_p = head_pad(bk[None, :])
    # masked q weights: one M-tile per (h2, hl) with only strip hl nonzero,
    # so the q output is block-diagonal across head strips (the scores
    # contraction is kept dense at partition base 0).
    qm = np.zeros((192, 8, 128))
    bqm = np.zeros((1, 8, 128))
    for h2 in range(2):
        for hl in range(4):
            s = slice(32 * hl, 32 * hl + HD)
            qm[:, 4 * h2 + hl, s] = wq_p[:, h2, s]
            bqm[:, 4 * h2 + hl, s] = bq_p[:, h2, s]
    qm_dr = np.zeros((97, 2, 8, 128))
    qm_dr[0:96, 0] = qm[0:96]
    qm_dr[96, 0] = bqm[0]
    qm_dr[0:96, 1] = qm[96:192]
    kk_dr = np.zeros((97, 2, 2, 128))
    kk_dr[0:96, 0] = wk_p[0:96]
    kk_dr[96, 0] = bk_p[0]
    kk_dr[0:96, 1] = wk_p[96:192]

    # v columns padded to 32-per-head strips (AV lhsT M must be 32-aligned)
    wv_t = np.zeros((C, 256))
    bv_p = np.zeros((1, 256))
    for h in range(HEADS):
        wv_t[:, 32 * h:32 * h + HD] = wv.T[:, HD * h:HD * (h + 1)]
        bv_p[0, 32 * h:32 * h + HD] = bv[HD * h:HD * (h + 1)]
    bv_p[0, 24] = 1.0    # exact-ones output column (proj bias carrier)
    bv_p[0, 128 + 24] = 1.0
    wv_dr = np.zeros((97, 2, 256))
    wv_dr[0:96, 0] = wv_t[0:96]
    wv_dr[96, 0] = bv_p[0]
    wv_dr[0:96, 1] = wv_t[96:192]

    # proj lhsT with K = padded O rows (strip layout); the two 128-row strip
    # chunks become the DoubleRow pair.  Row 24 of chunk0 carries proj_b
    # (O_cf row 24 is the exact-ones column).
    wp_t = i["proj_w"].astype(np.float64).T               # [cin, cout]
    wpk = np.zeros((2, 128, C))
    for h2 in range(2):
        for hl in range(4):
            wpk[h2, 32 * hl:32 * hl + HD, :] = \
                wp_t[96 * h2 + HD * hl:96 * h2 + HD * (hl + 1), :]
    wpk[0, 24, :] = i["proj_b"].astype(np.float64)
    wpk_dr = np.zeros((128, 2, 2, 96))
    for m in range(2):
        wpk_dr[:, 0, m] = wpk[0][:, 96 * m:96 * (m + 1)]
        wpk_dr[:, 1, m] = wpk[1][:, 96 * m:96 * (m + 1)]

    w1 = i["fc1_w"].astype(np.float64) * g2[None, :]
    b1f = i["fc1_w"].astype(np.float64) @ b2 + i["fc1_b"].astype(np.float64)
    w1_t = w1.T                                           # [192, 768]
    w1_dr = np.zeros((97, 2, HID))
    w1_dr[0:96, 0] = w1_t[0:96]
    w1_dr[96, 0] = b1f
    w1_dr[0:96, 1] = w1_t[96:192]

    w2_t = i["fc2_w"].astype(np.float64).T.reshape(6, 128, C).transpose(1, 0, 2)
    w2_dr = np.zeros((128, 3, 2, 2, 96))
    for pr in range(3):
        for t in range(2):
            for m in range(2):
                w2_dr[:, pr, t, m] = w2_t[:, 2 * pr + t, 96 * m:96 * (m + 1)]
    w2bias = i["fc2_b"].astype(np.float64).reshape(2, 96).T  # [96, 2]

    dw = i["dw_w"].astype(np.float64).reshape(HID, 9)     # [768, (ki kj)]
    dwdiag_dr = np.zeros((128, 6, 5, 2, 128))
    for ch in range(6):
        for pi, (t0, t1) in enumerate(DW_PAIRS):
            np.fill_diagonal(dwdiag_dr[:, ch, pi, 0, :],
                             dw[ch * 128:(ch + 1) * 128, t0])
            if pi < 4:
                np.fill_diagonal(dwdiag_dr[:, ch, pi, 1, :],
                                 dw[ch * 128:(ch + 1) * 128, t1])
    dwb = i["dw_b"].astype(np.float64).reshape(6, 128).T  # [128, 6]

    rel = _rel_pos_index(PS)                              # [49, 49]
    bias = i["rpe_table"].astype(np.float64)[rel.reshape(-1)].reshape(N, N, HEADS)
    # rpeb[m, (h, n)] = bias[h][n, m], added into the scores psum via an
    # identity matmul (pad rows 0)
    rpeb = np.zeros((64, HEADS * N))
    for h in range(HEADS):
        rpeb[0:49, h * N:(h + 1) * N] = bias[:, :, h].T   # [m, n]

    # all-ones columns over the valid 49 keys: the denominator matmul
    # replicates the softmax denominator across all 64 output partitions
    onesb = np.zeros((64, 64))
    onesb[0:49, :] = 1.0

    consts = {
        "qm_dr": qm_dr * WS, "kk_dr": kk_dr * WS, "wv_dr": wv_dr * WS,
        "wpk_dr": wpk_dr * WS, "w1_dr": w1_dr * WS, "w2_dr": w2_dr * WS,
        "dwdiag_dr": dwdiag_dr * WS,
        "rpeb": rpeb, "onesb": onesb,
        "ident": np.eye(128), "dwb": dwb, "w2bias": w2bias,
    }
    out = {}
    for k, v in consts.items():
        dt = CONST_SPECS[k][1]
        out[k] = v.astype(np.float32).astype(mybir.dt.np(dt))
    return out


CONST_SPECS = {
    "qm_dr": ([97, 2, 8, 128], fp8), "kk_dr": ([97, 2, 2, 128], fp8),
    "wv_dr": ([97, 2, 256], fp8), "wpk_dr": ([128, 2, 2, 96], fp8),
    "w1_dr": ([97, 2, 768], fp8), "w2_dr": ([128, 3, 2, 2, 96], fp8),
    "dwdiag_dr": ([128, 6, 5, 2, 128], fp8),
    "rpeb": ([64, 392], bf16), "onesb": ([64, 64], bf16),
    "ident": ([128, 128], bf16), "dwb": ([128, 6], f32),
    "w2bias": ([96, 2], f32),
}


def build_nc():
    nc = bass.Bass("TRN2", target_bir_lowering=False, debug=False)

    x = nc.dram_tensor("x", [T, C], f32, kind="ExternalInput")
    out = nc.dram_tensor("out", [T, C], f32, kind="ExternalOutput")
    x1d = nc.dram_tensor("x1d", [T, C], f32)

    cdram = {}
    for name, (shape, dt) in CONST_SPECS.items():
        cdram[name] = nc.dram_tensor(name, shape, dt, kind="ExternalInput")

    from contextlib import ExitStack

    with tile.TileContext(nc) as tc, ExitStack() as ctx:
        cpool = ctx.enter_context(tc.tile_pool(name="consts", bufs=1))
        cs = {}
        for name, (shape, dt) in CONST_SPECS.items():
            cs[name] = cpool.tile(shape, dt, tag=name, name=name)
            nc.sync.dma_start(out=cs[name][...], in_=cdram[name][...])
        eps_t = cpool.tile([112, 1], f32, tag="eps")
        nc.gpsimd.memset(eps_t[...], EPS)

        p2 = ctx.enter_context(tc.tile_pool(name="p2", bufs=2))
        p3 = ctx.enter_context(tc.tile_pool(name="p3", bufs=3))
        p4 = ctx.enter_context(tc.tile_pool(name="p4", bufs=4))
        p8 = ctx.enter_context(tc.tile_pool(name="p8", bufs=8))
        p16 = ctx.enter_context(tc.tile_pool(name="p16", bufs=16))
        mm = ctx.enter_context(tc.tile_pool(name="mm", bufs=4, space="PSUM"))
        tp = ctx.enter_context(tc.tile_pool(name="tp", bufs=2, space="PSUM"))
        dnp = ctx.enter_context(tc.tile_pool(name="dnp", bufs=2, space="PSUM"))

        # DRAM views
        x_v = x[...].rearrange("(b r c) d -> b c r d", b=NBLK, r=PS, c=WW)
        x1_v = x1d[...].rearrange("(b r c) d -> b c r d", b=NBLK, r=PS, c=WW)
        x1_rows = x1d[...].rearrange("(rr c) d -> c rr d", c=WW)  # [112,112,192]
        out_v = out[...].rearrange("(b r c) d -> b c r d", b=NBLK, r=PS, c=WW)

        _nblk = int(os.environ.get("BASSK_NBLK", NBLK))
        _stages = os.environ.get("BASSK_STAGES", "ab")
        # ------------------------------------------------------------------
        # Stage A: LN1 + window attention + proj + residual -> x1d
        # ------------------------------------------------------------------
        for blk in range(_nblk if "a" in _stages else 0):
            x_t = p2.tile([112, PS, C], f32, tag="x_t", bufs=3)
            nc.sync.dma_start(out=x_t[...], in_=x_v[blk])

            # LN1 (tokens on partitions: partition=col, free=(row, ch))
            mv = p2.tile([112, PS, 2], f32, tag="mv")
            for r in range(PS):
                st = p2.tile([112, 6], f32, tag="st")
                nc.vector.bn_stats(out=st[...], in_=x_t[:, r, :])
                nc.vector.bn_aggr(out=mv[:, r, :], in_=st[...])
            sd = p2.tile([112, PS], f32, tag="sd")
            nc.scalar.activation(out=sd[...], in_=mv[:, :, 1], func=AF.Sqrt,
                                 bias=eps_t[...], scale=1.0)
            rstd = p2.tile([112, PS], f32, tag="rstd")
            nc.vector.reciprocal(out=rstd[...], in_=sd[...])
            xn = p2.tile([112, PS, C], bf16, tag="xn", bufs=3)
            for r in range(PS):
                eng = nc.vector if r % 2 == 0 else nc.gpsimd
                eng.tensor_scalar(out=xn[:, r, :], in0=x_t[:, r, :],
                                  scalar1=mv[:, r, 0:1], scalar2=rstd[:, r:r + 1],
                                  op0=AL.subtract, op1=AL.mult)

            # transpose to channel-first fp8, evacuate window-major into the
            # DoubleRow 2-tile layout: cf[c, chunk, 49*w + n], n = 7*r + cl
            cf = p4.tile([97, 2, BT + 16], fp8, tag="xn_cf", bufs=3)
            for h in range(2):
                for rg, (r0, nr) in enumerate([(0, 4), (4, 3)]):
                    pt = tp.tile([96, 112 * nr], bf16, tag="tp", padded_shape=[96, 1024])
                    for j in range(nr):
                        nc.tensor.transpose(pt[:, 112 * j:112 * (j + 1)],
                                            xn[:, r0 + j, 96 * h:96 * (h + 1)],
                                            cs["ident"][0:112, 0:112])
                    dst = cf[0:96, h, 0:BT].rearrange("p (w r c) -> p r w c",
                                                      w=NW, r=PS, c=PS)
                    eng = nc.vector if rg == 0 else nc.scalar
                    if rg == 0:
                        eng.tensor_copy(out=dst[:, r0:r0 + nr], in_=pt[...])
                    else:
                        eng.copy(out=dst[:, r0:r0 + nr], in_=pt[...])
            nc.gpsimd.memset(cf[96:97, 0, :], 1.0)
            nc.gpsimd.memset(cf[96:97, 1, :], 0.0)
            nc.gpsimd.memset(cf[0:96, :, BT:BT + 16], 0.0)

            # k channel-first (head strips padded to 32, zeros in pads)
            k_sb = []
            for h2 in range(2):
                ksb = p8.tile([128, BT + 16], bf16, tag="qk_sb", bufs=4)
                k_sb.append(ksb)
                for nh in range(2):
                    ps_t = mm.tile([128, 392], f32, tag="mm",
                                   padded_shape=[128, 512])
                    nc.tensor.matmul(ps_t[...], cs["kk_dr"][:, :, h2, :],
                                     cf[0:97, :, 392 * nh:392 * (nh + 1)],
                                     start=True, stop=True, perf_mode=DR)
                    if nh == 0:
                        nc.vector.tensor_scalar(out=ksb[:, 0:392], in0=ps_t[...],
                                                scalar1=WSI, scalar2=None,
                                                op0=AL.mult)
                    else:
                        nc.scalar.activation(out=ksb[:, 392:784], in_=ps_t[...],
                                             func=AF.Copy, scale=WSI)
                nc.gpsimd.memset(ksb[:, BT:BT + 16], 0.0)

            # q block-diagonal: q_bd[h2][:, hl, t] nonzero only in strip hl
            q_bd = []
            qev = 0
            for h2 in range(2):
                qb = p4.tile([128, 4, BT], bf16, tag=f"q_bd{h2}", bufs=2)
                q_bd.append(qb)
                for hl in range(4):
                    for nh in range(2):
                        ps_t = mm.tile([128, 392], f32, tag="mm",
                                       padded_shape=[128, 512])
                        nc.tensor.matmul(ps_t[...], cs["qm_dr"][:, :, 4 * h2 + hl, :],
                                         cf[0:97, :, 392 * nh:392 * (nh + 1)],
                                         start=True, stop=True, perf_mode=DR)
                        dst = qb[:, hl, 392 * nh:392 * (nh + 1)]
                        if qev % 2 == 0:
                            nc.vector.tensor_scalar(out=dst, in0=ps_t[...],
                                                    scalar1=WSI, scalar2=None,
                                                    op0=AL.mult)
                        else:
                            nc.scalar.activation(out=dst, in_=ps_t[...],
                                                 func=AF.Copy, scale=WSI)
                        qev += 1

            # v token-major per pair: [64, (wl, 32-padded heads)] at base 0
            v_sb = []
            for p in range(8):
                vp = mm.tile([64, 512], f32, tag="mm", padded_shape=[64, 512])
                for wl in range(2):
                    w = 2 * p + wl
                    nc.tensor.matmul(vp[0:64, 256 * wl:256 * (wl + 1)],
                                     cf[0:97, :, N * w:N * w + 64],
                                     cs["wv_dr"][...], start=True, stop=True,
                                     perf_mode=DR)
                vs = p16.tile([64, 512], bf16, tag="v_sb", bufs=10)
                v_sb.append(vs)
                if p % 2 == 0:
                    nc.vector.tensor_scalar(out=vs[...], in0=vp[...],
                                            scalar1=WSI, scalar2=None,
                                            op0=AL.mult)
                else:
                    nc.scalar.activation(out=vs[...], in_=vp[...],
                                         func=AF.Copy, scale=WSI)

            # scores + softmax per (pair, window): St_wl [64, (h, n)]
            # 4 heads per matmul: rhs spans the q_bd hl dimension.
            ste = []
            for p in range(8):
                se_pair = []
                for wl in range(2):
                    w = 2 * p + wl
                    st_ps = mm.tile([64, 392], f32, tag="mm",
                                    padded_shape=[64, 512])
                    for h2 in range(2):
                        nc.tensor.matmul(
                            st_ps[0:64, 196 * h2:196 * (h2 + 1)],
                            k_sb[h2][0:128, N * w:N * w + 64],
                            q_bd[h2][:, :, N * w:N * (w + 1)],
                            start=True, stop=False)
                    nc.tensor.matmul(st_ps[...], cs["ident"][0:64, 0:64],
                                     cs["rpeb"][...], start=False, stop=True)
                    se = p16.tile([64, 392], bf16, tag="ste", bufs=16)
                    se_pair.append(se)
                    nc.scalar.activation(out=se[...], in_=st_ps[...],
                                         func=AF.Exp)
                    dnb = dnp.tile([64, 392], f32, tag="dnp",
                                   padded_shape=[64, 512])
                    nc.tensor.matmul(dnb[...], cs["onesb"][...], se[...],
                                     start=True, stop=True)
                    rb = p8.tile([64, 392], bf16, tag="rb", bufs=6)
                    with nc.allow_low_precision(reason="bf16 softmax denom, matches baseline"):
                        nc.vector.reciprocal(out=rb[0:49, :], in_=dnb[0:49, :])
                    nc.gpsimd.tensor_tensor(out=se[0:49, :], in0=se[0:49, :],
                                            in1=rb[0:49, :], op=AL.mult)
                ste.append(se_pair)

            # AV -> O channel-first fp8, heads in 32-strips (window-major)
            oc = p4.tile([128, 2, BT], fp8, tag="o_cf", bufs=2)
            for h2 in range(2):
                for bank, (p0, npair) in enumerate([(0, 5), (5, 3)]):
                    op_ps = mm.tile([128, 98 * npair], f32, tag="mm",
                                    padded_shape=[128, 512])
                    for pl in range(npair):
                        p = p0 + pl
                        for hl in range(4):
                            h = 4 * h2 + hl
                            for wl in range(2):
                                nc.tensor.matmul(
                                    op_ps[32 * hl:32 * hl + 32,
                                          98 * pl + N * wl:98 * pl + N * (wl + 1)],
                                    v_sb[p][0:49, 256 * wl + 32 * h:
                                            256 * wl + 32 * (h + 1)],
                                    ste[p][wl][0:49, N * h:N * (h + 1)],
                                    start=True, stop=True,
                                    tile_position=(0, 32 * hl))
                    dst = oc[:, h2, 98 * p0:98 * (p0 + npair)]
                    if bank == 0:
                        nc.vector.tensor_copy(out=dst, in_=op_ps[...])
                    else:
                        nc.scalar.copy(out=dst, in_=op_ps[...])

            # proj (+bias via row 24 of wpk chunk0) -> token-major evac
            x1_t = p2.tile([112, PS, C], f32, tag="x1_t", bufs=3)
            for m in range(2):
                pr_sb = p4.tile([96, BT], bf16, tag="pr_sb", bufs=2)
                for nh in range(2):
                    pp = mm.tile([96, 392], f32, tag="mm", padded_shape=[96, 512])
                    nc.tensor.matmul(pp[...], cs["wpk_dr"][:, :, m, :],
                                     oc[0:128, :, 392 * nh:392 * (nh + 1)],
                                     start=True, stop=True, perf_mode=DR)
                    # permute window-major -> token-major on the evac
                    dst = pr_sb[:, :].rearrange("p (r w c) -> p w r c",
                                                r=PS, w=NW, c=PS)
                    src = pp[:, :].rearrange("p (w r c) -> p w r c",
                                             w=8, r=PS, c=PS)
                    if nh == 0:
                        nc.vector.tensor_scalar(out=dst[:, 8 * nh:8 * (nh + 1)],
                                                in0=src, scalar1=WSI,
                                                scalar2=None, op0=AL.mult)
                    else:
                        nc.scalar.activation(out=dst[:, 8 * nh:8 * (nh + 1)],
                                             in_=src, func=AF.Copy, scale=WSI)
                # transpose back + residual add
                for rg, (r0, nr) in enumerate([(0, 4), (4, 3)]):
                    ptt = tp.tile([112, 96 * nr], bf16, tag="tp", padded_shape=[112, 1024])
                    for j in range(nr):
                        nc.tensor.transpose(ptt[:, 96 * j:96 * (j + 1)],
                                            pr_sb[:, 112 * (r0 + j):112 * (r0 + j + 1)],
                                            cs["ident"][0:96, 0:96])
                    src = ptt[:, :].rearrange("p (j d) -> p j d", j=nr)
                    dst = x1_t[:, r0:r0 + nr, 96 * m:96 * (m + 1)]
                    ins = x_t[:, r0:r0 + nr, 96 * m:96 * (m + 1)]
                    nc.vector.tensor_tensor(out=dst, in0=src, in1=ins, op=AL.add)
            nc.sync.dma_start(out=x1_v[blk], in_=x1_t[...])

        # ------------------------------------------------------------------
        # Stage B: LN2 + fc1 + dwconv + gelu + fc2 + residual -> out
        # ------------------------------------------------------------------
        for blk in range(_nblk if "b" in _stages else 0):
            r_lo = blk * PS - 1
            x1h = p2.tile([112, 9, C], f32, tag="x1h", bufs=3)
            s0 = 1 if blk == 0 else 0
            s1 = 8 if blk == NBLK - 1 else 9
            nc.sync.dma_start(out=x1h[:, s0:s1, :],
                              in_=x1_rows[:, r_lo + s0:r_lo + s1, :])
            if blk == 0:
                nc.gpsimd.memset(x1h[:, 0, :], 0.0)
            if blk == NBLK - 1:
                nc.gpsimd.memset(x1h[:, 8, :], 0.0)

            mv2 = p2.tile([112, 9, 2], f32, tag="mv2")
            for r in range(9):
                st = p2.tile([112, 6], f32, tag="st")
                nc.vector.bn_stats(out=st[...], in_=x1h[:, r, :])
                nc.vector.bn_aggr(out=mv2[:, r, :], in_=st[...])
            sd2 = p2.tile([112, 9], f32, tag="sd2")
            nc.scalar.activation(out=sd2[...], in_=mv2[:, :, 1], func=AF.Sqrt,
                                 bias=eps_t[...], scale=1.0)
            rstd2 = p2.tile([112, 9], f32, tag="rstd2")
            nc.vector.reciprocal(out=rstd2[...], in_=sd2[...])
            xn2 = p2.tile([112, 9, C], bf16, tag="xn2", bufs=3)
            for r in range(9):
                eng = nc.vector if r % 2 == 0 else nc.gpsimd
                eng.tensor_scalar(out=xn2[:, r, :], in0=x1h[:, r, :],
                                  scalar1=mv2[:, r, 0:1], scalar2=rstd2[:, r:r + 1],
                                  op0=AL.subtract, op1=AL.mult)

            cf2 = p2.tile([97, 2, 9 * 112], fp8, tag="xn2_cf", bufs=3)
            for h in range(2):
                for rg in range(3):
                    pt = tp.tile([96, 336], bf16, tag="tp", padded_shape=[96, 1024])
                    for j in range(3):
                        r = 3 * rg + j
                        nc.tensor.transpose(pt[:, 112 * j:112 * (j + 1)],
                                            xn2[:, r, 96 * h:96 * (h + 1)],
                                            cs["ident"][0:112, 0:112])
                    eng = [nc.vector.tensor_copy, nc.scalar.copy,
                           nc.vector.tensor_copy][rg]
                    eng(out=cf2[0:96, h, 336 * rg:336 * (rg + 1)], in_=pt[...])
            nc.gpsimd.memset(cf2[96:97, 0, :], 1.0)
            nc.gpsimd.memset(cf2[96:97, 1, :], 0.0)

            # fc1 -> h fp8 (with 1-col zero padding for the conv), then
            # depthwise conv as 5 accumulating DoubleRow tap-pair matmuls,
            # gelu (+bias, /32) -> g fp8 [128, ch, BT]
            g_sb = p3.tile([128, 6, BT], fp8, tag="g_sb")
            for ch in range(6):
                h_sb = p3.tile([128, 9, 114], fp8, tag="h_sb")
                for g in range(3):
                    hp = mm.tile([128, 336], f32, tag="mm", padded_shape=[128, 512])
                    nc.tensor.matmul(hp[...],
                                     cs["w1_dr"][:, :, 128 * ch:128 * (ch + 1)],
                                     cf2[0:97, :, 336 * g:336 * (g + 1)],
                                     start=True, stop=True, perf_mode=DR)
                    dst = h_sb[:, 3 * g:3 * (g + 1), 1:113]
                    if g == 0:
                        nc.vector.tensor_scalar(out=dst, in0=hp[...],
                                                scalar1=WSI, scalar2=None,
                                                op0=AL.mult)
                    elif g <= 1:
                        nc.scalar.activation(out=dst, in_=hp[...],
                                             func=AF.Copy, scale=WSI)
                    else:
                        nc.vector.tensor_scalar(out=dst, in0=hp[...],
                                                scalar1=WSI, scalar2=None,
                                                op0=AL.mult)
                # zero the padding columns (and halo rows at image edges)
                pad = h_sb[:, :, :].rearrange("p r c -> p (r c)")
                nc.gpsimd.memset(
                    bass.AP(tensor=pad.tensor, offset=pad.offset,
                            ap=[pad.ap[0], [114, 9], [113, 2]]), 0.0)
                if blk == 0:
                    nc.gpsimd.memset(h_sb[:, 0, :], 0.0)
                if blk == NBLK - 1:
                    nc.gpsimd.memset(h_sb[:, 8, :], 0.0)

                for bank, (r0, nr) in enumerate([(1, 4), (5, 3)]):
                    cp = mm.tile([128, 112 * nr], f32, tag="mm", padded_shape=[128, 512])
                    for pi, (t0, t1) in enumerate(DW_PAIRS):
                        o0 = _tap_off(t0)
                        delta = _tap_off(t1) - o0
                        base = h_sb[:, r0:r0 + nr, 1:113]
                        rhs = bass.AP(tensor=base.tensor,
                                      offset=base.offset + o0,
                                      ap=[base.ap[0], [delta, 2],
                                          [114, nr], [1, 112]])
                        nc.tensor.matmul(cp[...],
                                         cs["dwdiag_dr"][:, ch, pi, :, :],
                                         rhs, start=(pi == 0), stop=(pi == 4),
                                         perf_mode=DR)
                    nc.scalar.activation(out=g_sb[:, ch, 112 * (r0 - 1):112 * (r0 - 1 + nr)],
                                         in_=cp[...], func=AF.Gelu,
                                         bias=cs["dwb"][:, ch:ch + 1], scale=WSI)

            # fc2 DoubleRow (+bias via the evacuation) -> token-major -> out
            out_t = p2.tile([112, PS, C], f32, tag="out_t", bufs=3)
            for m in range(2):
                f2_sb = p4.tile([96, BT], bf16, tag="f2_sb", bufs=2)
                for nh in range(2):
                    fp = mm.tile([96, 392], f32, tag="mm", padded_shape=[96, 512])
                    for pr in range(3):
                        nc.tensor.matmul(fp[...], cs["w2_dr"][:, pr, :, m, :],
                                         g_sb[0:128, 2 * pr:2 * pr + 2,
                                              392 * nh:392 * (nh + 1)],
                                         start=(pr == 0), stop=(pr == 2),
                                         perf_mode=DR)
                    dst = f2_sb[:, 392 * nh:392 * (nh + 1)]
                    if nh == 0:
                        nc.vector.tensor_scalar(
                            out=dst, in0=fp[...], scalar1=WSI,
                            scalar2=cs["w2bias"][:, m:m + 1],
                            op0=AL.mult, op1=AL.add)
                    else:
                        nc.vector.tensor_scalar(
                            out=dst, in0=fp[...], scalar1=WSI,
                            scalar2=cs["w2bias"][:, m:m + 1],
                            op0=AL.mult, op1=AL.add)
                for rg, (r0, nr) in enumerate([(0, 4), (4, 3)]):
                    ptt = tp.tile([112, 96 * nr], bf16, tag="tp", padded_shape=[112, 1024])
                    for j in range(nr):
                        nc.tensor.transpose(ptt[:, 96 * j:96 * (j + 1)],
                                            f2_sb[:, 112 * (r0 + j):112 * (r0 + j + 1)],
                                            cs["ident"][0:96, 0:96])
                    src = ptt[:, :].rearrange("p (j d) -> p j d", j=nr)
                    dst = out_t[:, r0:r0 + nr, 96 * m:96 * (m + 1)]
                    ins = x1h[:, 1 + r0:1 + r0 + nr, 96 * m:96 * (m + 1)]
                    nc.vector.tensor_tensor(out=dst, in0=src, in1=ins, op=AL.add)
            nc.sync.dma_start(out=out_v[blk], in_=out_t[...])

    _split_excess_waits(nc)
    return nc


_BUILT = None


def _get_nc():
    global _BUILT
    if _BUILT is None:
        _BUILT = build_nc()
    return _BUILT


def kernel(**inputs):
    x = np.asarray(inputs["x"], dtype=np.float32)
    assert x.shape == (B, T, C)
    consts = prep_consts(inputs)
    nc = _get_nc()
    in_maps = []
    for c in range(B):
        m = {"x": np.ascontiguousarray(x[c])}
        m.update(consts)
        in_maps.append(m)
    res = run_bass_kernel_spmd(nc, in_maps, list(range(8)))
    return np.stack([res.results[c]["out"] for c in range(B)], axis=0)


if __name__ == "__main__":
    # quick shape smoke: build only
    nc = _get_nc()
    print("built ok")
